# revision 14
# baseline (speedup 1.0000x reference)
"""BoundaryAwareSmoothAttention Trainium2 kernel (v2).

Math (per batch b, HW=4096, C=64):
  Q = Wq x, K = Wk x, V = Wv x                   (1x1 convs, biases zero)
  S[n,m]  = q_n . k_m
  edge[m] = sigmoid(We2 . relu(BN(conv3x3(x))) + be2)
  mod[m]  = 1 + beta*edge[m]
  fa[n,m] = exp(S[n,m] - 32 + ln mod[m])         (mod folded into the exp
                                                  bias; softmax Z cancels in
                                                  the L1 renorm)
  out[c,n] = gamma * (sum_m V[c,m] fa[n,m]) / (sum_m fa[n,m]) + x[c,n]

Sharding: 8 cores = 4 batches x 2 query-halves (n in [h*2048, h*2048+2048)).

v2 vs v1: the AV product is computed TRANSPOSED -- for each 128-query chunk
j, P_j[n,c] = sum_m fa[m,n]*VT[m,c] with fa (bf16) as the stationary
operand, so all 128 psum partitions are live (v1's orientation used 66 of
128): AV drops 65536 -> ~34K PE cycles.  VT holds [gamma*V^T | 1 | 0] in
bf16 and the ones column accumulates the L1 denominator for free.  V^T is
projected in bf16 (1 cy/row at free-dim 64 where f32r pays 4).  The exp
stream is split column-wise across ACT (native exp, psum->bf16) and DVE
(Schraudolph: bf16 bits = rint(C1*S + PB[m]) written as int16), halving the
66us single-engine exp stream and letting the 2-buffer S psum drain fast.
The epilogue is one DVE scalar_tensor_tensor per chunk in the transposed
layout (the denominator is per-partition there); the host untransposes.
A 1x1 warmup matmul at t~0 starts the PE p-state ramp before DMAs land.
"""

import numpy as np
import ml_dtypes

import concourse.bass as bass
import concourse.tile as tile
from concourse import bacc, mybir
from concourse.bass_utils import run_bass_kernel_spmd

F32 = mybir.dt.float32
F32R = mybir.dt.float32r
BF16 = mybir.dt.bfloat16
I16 = mybir.dt.int16
AF = mybir.ActivationFunctionType
ALU = mybir.AluOpType

C = 64
CH = 32
HW = 4096
NQ = 2048   # queries per core
QB = 1024   # queries per position-block
NPAIR = 16  # positions per qb; position p covers m-chunks 2p, 2p+1
NPOS = 32
NCH = 16    # 128-query output chunks per core
N_CORES = 8
SHIFT = 32.0
BN_EPS = 1e-5

# Schraudolph bf16 exp: bits_i16 = rint(SC1*x + SC2), calibrated for
# round-to-nearest f32->i16 conversion (max rel err 3.3%).  The DVE path
# uses its own softmax shift (cancels per query in the L1 renorm; each
# query column is written by exactly one engine) chosen so the int16 bits
# stay in [0, 32767] for any |S| <= ~70: S-16 in [-86, +54] maps to
# [~320, ~29300].
LOG2E = 1.4426950408889634
SC1 = 128.0 * LOG2E
SC2 = 128.0 * (127.0 - 0.044)
SHIFT_DVE = 16.0

CFG = {
    "lag": 3,        # AV trails QK by this many positions
    "act_cols": 640,  # per S tile: ACT exp cols (DVE takes QB - act_cols)
    "fa_bufs": 12,
}


def build_program(beta: float, gamma: float, be2: float):
    nc = bacc.Bacc("TRN2", target_bir_lowering=False, debug=False,
                   num_devices=N_CORES)

    def din(name, shape, dt=F32):
        return nc.dram_tensor(name, shape, dt, kind="ExternalInput").ap()

    x_d = din("x", [C, HW])
    xbf_d = din("x_bf", [C, HW], BF16)
    xq_d = din("xq", [C, NQ])
    xqt_d = din("xqt", [128, NCH * 64])
    wq_d = din("wq_t", [C, C])
    wk_d = din("wk_t", [C, C])
    wv_d = din("wv_bf", [C, C], BF16)     # gamma * Wv^T, bf16
    wcatA_d = din("wcatA", [128, 96])
    wcatB_d = din("wcatB", [64, 96])
    we2_d = din("we2_t", [CH, 2])
    bnt_d = din("bn_t", [CH, 1])
    out_d = nc.dram_tensor("out", [128, NCH * 64], F32,
                           kind="ExternalOutput").ap()
    import os
    DBG = os.environ.get("KDBG", "")
    dbg_d = {}
    if DBG:
        dbg_d["vt"] = nc.dram_tensor("dbg_vt", [128, 66 * 32], BF16,
                                     kind="ExternalOutput").ap()
        dbg_d["lnm"] = nc.dram_tensor("dbg_lnm", [128, 32], F32,
                                      kind="ExternalOutput").ap()
        dbg_d["fa0"] = nc.dram_tensor("dbg_fa0", [128, QB], BF16,
                                      kind="ExternalOutput").ap()
        dbg_d["fa1"] = nc.dram_tensor("dbg_fa1", [128, QB], BF16,
                                      kind="ExternalOutput").ap()
        dbg_d["av0"] = nc.dram_tensor("dbg_av0", [128, 264], F32,
                                      kind="ExternalOutput").ap()
        dbg_d["av3"] = nc.dram_tensor("dbg_av3", [128, 264], F32,
                                      kind="ExternalOutput").ap()
        dbg_d["fa14"] = nc.dram_tensor("dbg_fa14", [128, QB], BF16,
                                       kind="ExternalOutput").ap()
        dbg_d["k2"] = nc.dram_tensor("dbg_k2", [128, HW], F32,
                                     kind="ExternalOutput").ap()
        dbg_d["q2"] = nc.dram_tensor("dbg_q2", [128, NQ], F32,
                                     kind="ExternalOutput").ap()

    ACOL = CFG["act_cols"]
    LAG = CFG["lag"]

    with tile.TileContext(nc) as tc:
        with (
            tc.tile_pool(name="consts", bufs=1) as consts,
            tc.tile_pool(name="bigs", bufs=1) as bigs,
            tc.tile_pool(name="fa_p", bufs=CFG["fa_bufs"]) as fa_p,
            tc.tile_pool(name="ep", bufs=4) as ep,
            tc.tile_pool(name="ps_s", bufs=2, space="PSUM") as ps_s,
            tc.tile_pool(name="ps_av", bufs=2, space="PSUM") as ps_av,
            tc.tile_pool(name="ps_bg", bufs=2, space="PSUM") as ps_bg,
        ):
            # ---- big SBUF tensors -----------------------------------------
            A = bigs.tile([128, HW], F32R)       # [x_m1 ; x]
            B = bigs.tile([64, HW], F32R)        # x_p1
            x_r = bigs.tile([C, HW], F32R)       # K proj rhs
            x_bf = bigs.tile([C, HW], BF16)      # V^T proj lhsT
            xq_r = bigs.tile([C, NQ], F32R)      # Q proj rhs
            xqT = bigs.tile([128, NCH * 64], F32)
            K2 = bigs.tile([128, HW], F32R)      # K duplicated on both halves
            Q2 = bigs.tile([128, NQ], F32R)      # Q duplicated on both halves
            VT = bigs.tile([128, 66 * 32], BF16)  # chunks [gamma*V^T | 1 | 0]
            relu_sb = bigs.tile([CH, HW], F32R)
            out_sb = bigs.tile([128, NCH * 64], F32)
            VT_v = VT[:].rearrange("p (j w) -> p j w", w=66)

            # ---- constant/small tiles -------------------------------------
            wq_r = consts.tile([C, C], F32R)
            wk_r = consts.tile([C, C], F32R)
            wv_b = consts.tile([C, C], BF16)
            wcatA_r = consts.tile([128, 96], F32R)
            wcatB_r = consts.tile([64, 96], F32R)
            we2_r = consts.tile([CH, 2], F32R)
            bnt_sb = consts.tile([CH, 1], F32)
            ones_b = consts.tile([128, 1], F32)
            ones32 = consts.tile([128, 32], F32)
            z32 = consts.tile([128, 32], F32)
            z64 = consts.tile([64, 64], F32)
            warm = consts.tile([1, 1], BF16)
            lnm_sb = consts.tile([128, 32], F32)   # ln(mod) per m-chunk col
            bia_sb = consts.tile([128, 32], F32)   # lnm - 32 (ACT exp bias)
            pb_sb = consts.tile([128, 32], F32)    # SC1*(lnm-32)+SC2 (DVE)

            # warmup matmul to start the PE p-state ramp at t~0
            nc.vector.memset(warm[:], 0.0)
            wps = ps_bg.tile([1, 1], F32, tag="bg")
            nc.tensor.matmul(wps[:], warm[:], warm[:], start=True, stop=True)

            # input DMAs, QK/conv path first
            wk_f = consts.tile([C, C], F32)
            wq_f = consts.tile([C, C], F32)
            nc.sync.dma_start(out=wk_f[:], in_=wk_d[:])
            nc.sync.dma_start(out=wq_f[:], in_=wq_d[:])
            nc.vector.tensor_copy(wk_r[:], wk_f[:])
            nc.vector.tensor_copy(wq_r[:], wq_f[:])
            nc.gpsimd.dma_start(out=x_r[:, 0:1152], in_=x_d[:, 0:1152])
            nc.gpsimd.dma_start(out=xq_r[:, 0:1024], in_=xq_d[:, 0:1024])
            nc.gpsimd.dma_start(out=A[64:128, 0:1152], in_=x_d[:, 0:1152])
            nc.gpsimd.dma_start(out=wcatA_r[:], in_=wcatA_d[:])
            nc.gpsimd.dma_start(out=wcatB_r[:], in_=wcatB_d[:])
            nc.gpsimd.dma_start(out=we2_r[:], in_=we2_d[:])
            nc.sync.dma_start(out=bnt_sb[:], in_=bnt_d[:])
            nc.gpsimd.dma_start(out=xq_r[:, 1024:NQ], in_=xq_d[:, 1024:NQ])
            nc.gpsimd.dma_start(out=x_r[:, 1152:HW], in_=x_d[:, 1152:HW])
            nc.gpsimd.dma_start(out=A[64:128, 1152:HW], in_=x_d[:, 1152:HW])
            nc.scalar.dma_start(out=wv_b[:], in_=wv_d[:])
            nc.scalar.dma_start(out=x_bf[:, 0:1024], in_=xbf_d[:, 0:1024])
            nc.scalar.dma_start(out=x_bf[:, 1024:HW], in_=xbf_d[:, 1024:HW])
            nc.scalar.dma_start(out=xqT[:], in_=xqt_d[:])

            nc.vector.memset(ones_b[:], 1.0)
            nc.vector.memset(ones32[:], 1.0)
            nc.vector.memset(z32[:], 0.0)
            nc.vector.memset(z64[:], 0.0)
            # VT col 64 = 1 (denominator), col 65 = 0 (pad)
            nc.vector.tensor_copy(VT_v[:, :, 64], ones32[:])
            nc.vector.tensor_copy(VT_v[:, :, 65], z32[:])

            # ---- Q/K projections ------------------------------------------
            def emit_kproj(t):
                kp = ps_bg.tile([64, 512], F32, tag="bg")
                nc.tensor.matmul(kp[:], wk_r[:], x_r[:, 512 * t:512 * t + 512],
                                 start=True, stop=True)
                nc.vector.tensor_copy(K2[0:64, 512 * t:512 * t + 512], kp[:])
                nc.sync.dma_start(out=K2[64:128, 512 * t:512 * t + 512],
                                  in_=K2[0:64, 512 * t:512 * t + 512])

            def emit_qproj(t):
                qp = ps_bg.tile([64, 512], F32, tag="bg")
                nc.tensor.matmul(qp[:], wq_r[:], xq_r[:, 512 * t:512 * t + 512],
                                 start=True, stop=True)
                nc.vector.tensor_copy(Q2[0:64, 512 * t:512 * t + 512], qp[:])
                nc.sync.dma_start(out=Q2[64:128, 512 * t:512 * t + 512],
                                  in_=Q2[0:64, 512 * t:512 * t + 512])

            # ---- edge/background pipeline ---------------------------------
            A_vw = A[0:64, :].rearrange("p (y x) -> p y x", x=64)
            B_vw = B[0:64, :].rearrange("p (y x) -> p y x", x=64)

            def bg_prep(t):
                # x_m1 (A rows 0-63) / x_p1 (B) slices this conv tile reads,
                # plus their SAME-pad zeros; on Pool to keep DVE free
                r0 = 0 if t == 0 else 512 * t + 576
                r1 = min(512 * t + 1088, HW)
                if r1 <= r0:
                    return
                a0 = max(r0, 1)
                nc.gpsimd.tensor_copy(A[0:64, a0:r1], x_r[:, a0 - 1:r1 - 1])
                nc.gpsimd.tensor_copy(B[0:64, r0:r1 - 1], x_r[:, r0 + 1:r1])
                y0, y1 = r0 // 64, r1 // 64
                nc.gpsimd.tensor_copy(A_vw[:, y0:y1, 0], z64[:, 0:y1 - y0])
                nc.gpsimd.tensor_copy(B_vw[:, y0:y1, 63], z64[:, 0:y1 - y0])

            def bg_front(t):
                bg_prep(t)
                t0 = 512 * t
                ep_ps = ps_bg.tile([CH, 512], F32, tag="bg")
                mms = []
                for dy in (0, -1, 1):
                    lo = max(t0, -64 * dy)
                    hi = min(t0 + 512, HW - max(0, 64 * dy))
                    if hi <= lo:
                        continue
                    sl_out = ep_ps[:, lo - t0:hi - t0]
                    ky = dy + 1
                    mms.append((sl_out, wcatA_r[:, 32 * ky:32 * ky + 32],
                                A[:, lo + 64 * dy:hi + 64 * dy]))
                    mms.append((sl_out, wcatB_r[:, 32 * ky:32 * ky + 32],
                                B[0:64, lo + 64 * dy:hi + 64 * dy]))
                for i, (o, l, r) in enumerate(mms):
                    nc.tensor.matmul(o, l, r, start=(i == 0),
                                     stop=(i == len(mms) - 1),
                                     skip_group_check=True)
                nc.scalar.activation(relu_sb[:, t0:t0 + 512], ep_ps[:],
                                     AF.Relu, bias=bnt_sb[:, 0:1], scale=1.0)
                eg_t = ps_bg.tile([128, 8], F32, tag="bg")
                for jj in range(4):
                    j = 4 * t + jj
                    nc.tensor.matmul(eg_t[:, 2 * jj:2 * jj + 2],
                                     relu_sb[:, 128 * j:128 * j + 128],
                                     we2_r[:], start=True, stop=True,
                                     skip_group_check=True)
                return eg_t

            def bg_back(t, eg_t):
                # sigma -> ln(1+beta*sigma) on ACT; exp-bias tables on Pool
                g = slice(4 * t, 4 * t + 4)
                sig = ep.tile([128, 4], F32, tag="sg")
                nc.scalar.activation(sig[:], eg_t[:, 0:8:2], AF.Sigmoid,
                                     bias=be2, scale=1.0)
                nc.scalar.activation(lnm_sb[:, g], sig[:], AF.Ln,
                                     bias=ones_b[:, 0:1], scale=beta)
                nc.gpsimd.tensor_scalar_add(bia_sb[:, g], lnm_sb[:, g], -SHIFT)
                nc.gpsimd.tensor_scalar(out=pb_sb[:, g], in0=lnm_sb[:, g],
                                        scalar1=SC1,
                                        scalar2=SC2 - SHIFT_DVE * SC1,
                                        op0=ALU.mult, op1=ALU.add)

            def vt_mms(t):
                # gamma*V^T for m-chunks 4t..4t+3, bf16 (full rate at free=64)
                for jj in (0, 2):
                    j = 4 * t + jj
                    vp = ps_bg.tile([128, 128], F32, tag="bg")
                    nc.tensor.matmul(vp[:, 0:64], x_bf[:, 128 * j:128 * j + 128],
                                     wv_b[:], start=True, stop=True,
                                     skip_group_check=True)
                    nc.tensor.matmul(vp[:, 64:128],
                                     x_bf[:, 128 * (j + 1):128 * (j + 1) + 128],
                                     wv_b[:], start=True, stop=True,
                                     skip_group_check=True)
                    vp_v = vp[:].rearrange("p (j w) -> p j w", w=64)
                    nc.scalar.activation(VT_v[:, j:j + 2, 0:64], vp_v[:],
                                         AF.Copy)

            # ---- QK + split exp -------------------------------------------
            fa_store = {}

            def emit_qk_half(pos, k):
                qb, pair = pos // NPAIR, pos % NPAIR
                q0 = QB * qb
                mc = 2 * pair + k
                r0, r1 = (0, 64) if k == 0 else (64, 128)
                s_ps = ps_s.tile([128, QB], F32, tag="s")
                for h in range(QB // 512):
                    nc.tensor.matmul(
                        s_ps[:, 512 * h:512 * h + 512],
                        K2[r0:r1, 128 * mc:128 * mc + 128],
                        Q2[r0:r1, q0 + 512 * h:q0 + 512 * h + 512],
                        start=True, stop=True)
                return mc, s_ps

            def emit_exp(qb, mc, s_ps):
                # column-split across ACT (native exp) and DVE (Schraudolph)
                fa = fa_p.tile([128, QB], BF16, tag="fa")
                if ACOL > 0:
                    nc.scalar.activation(fa[:, 0:ACOL], s_ps[:, 0:ACOL],
                                         AF.Exp, bias=bia_sb[:, mc:mc + 1],
                                         scale=1.0)
                if ACOL < QB:
                    nc.vector.tensor_scalar(
                        out=fa[:, ACOL:QB].bitcast(I16), in0=s_ps[:, ACOL:QB],
                        scalar1=SC1, scalar2=pb_sb[:, mc:mc + 1],
                        op0=ALU.mult, op1=ALU.add)
                if DBG and qb == 0 and mc in (0, 1):
                    nc.sync.dma_start(out=dbg_d["fa" + str(mc)][:], in_=fa[:])
                if DBG and qb == 1 and mc == 31:
                    nc.sync.dma_start(out=dbg_d["fa14"][:], in_=fa[:])
                fa_store[(qb, mc)] = fa

            # ---- AV (transposed) + epilogue -------------------------------
            av_tiles = {}

            def emit_av_half(pos_done, half):
                # apply fa pair (pos_done) to 4 of the 8 query-chunk
                # accumulators of its qb
                qb, pair = pos_done // NPAIR, pos_done % NPAIR
                if pair == 0 and half == 0:
                    tA = ps_av.tile([128, 264], F32, tag="av", name=f"avA{qb}")
                    tB = ps_av.tile([128, 264], F32, tag="av", name=f"avB{qb}")
                    # a start=True matmul would wipe the other in-flight
                    # accumulators sharing the bank, so zero via ACT and
                    # accumulate with start=False throughout
                    nc.scalar.memzero(tA[:])
                    nc.scalar.memzero(tB[:])
                    av_tiles[qb] = (tA, tB)
                t = av_tiles[qb][half]
                for jj in range(4):
                    j = 4 * half + jj
                    for mc in (2 * pair, 2 * pair + 1):
                        fa = fa_store[(qb, mc)]
                        nc.tensor.matmul(
                            t[:, 66 * jj:66 * jj + 66],
                            fa[:, 128 * j:128 * j + 128],
                            VT[:, 66 * mc:66 * mc + 66],
                            start=False, stop=(mc == 31),
                            skip_group_check=True)
                if half == 1:
                    del fa_store[(qb, 2 * pair)]
                    del fa_store[(qb, 2 * pair + 1)]

            def epilogue_half(qb, half):
                t = av_tiles[qb][half]
                if DBG and qb == 0 and half == 0:
                    avs = ep.tile([128, 264], F32, tag="dbgav")
                    nc.vector.tensor_copy(avs[:], t[:])
                    nc.sync.dma_start(out=dbg_d["av0"][:], in_=avs[:])
                if DBG and qb == 1 and half == 1:
                    avs = ep.tile([128, 264], F32, tag="dbgav")
                    nc.vector.tensor_copy(avs[:], t[:])
                    nc.sync.dma_start(out=dbg_d["av3"][:], in_=avs[:])
                tv = t[:].rearrange("p (j w) -> p j w", w=66)
                rc = ep.tile([128, 4], F32, tag="rc")
                nc.vector.reciprocal(rc[:], tv[:, :, 64])
                for jj in range(4):
                    k = 8 * qb + 4 * half + jj
                    nc.vector.scalar_tensor_tensor(
                        out=out_sb[:, 64 * k:64 * k + 64],
                        in0=tv[:, jj, 0:64], scalar=rc[:, jj:jj + 1],
                        in1=xqT[:, 64 * k:64 * k + 64],
                        op0=ALU.mult, op1=ALU.add)
                k0 = 8 * qb + 4 * half
                nc.sync.dma_start(out=out_d[:, 64 * k0:64 * k0 + 256],
                                  in_=out_sb[:, 64 * k0:64 * k0 + 256])

            # ---- main loop ------------------------------------------------
            # bg tile t must be ready before pos 2t (bias cols 4t..4t+3).
            eg0 = bg_front(0)
            bg_back(0, eg0)
            vt_mms(0)
            emit_kproj(0)
            emit_qproj(0)
            emit_qproj(1)
            eg1 = bg_front(1)
            bg_back(1, eg1)
            vt_mms(1)
            kproj_done, qproj_done, bg_done = 1, 2, 2
            av_next = 0

            def drain_av(pos, budget):
                nonlocal av_next
                while budget > 0 and av_next // 2 <= pos - LAG \
                        and av_next < 2 * NPOS:
                    emit_av_half(av_next // 2, av_next % 2)
                    if av_next % (2 * NPAIR) == 2 * NPAIR - 1:
                        qb = av_next // (2 * NPAIR)
                        epilogue_half(qb, 0)
                        epilogue_half(qb, 1)
                    av_next += 1
                    budget -= 1

            for pos in range(NPOS + LAG + 1):
                if pos < NPOS:
                    mc, s_ps = emit_qk_half(pos, 0)
                    emit_exp(pos // NPAIR, mc, s_ps)
                drain_av(pos, 1)
                if pos < NPOS:
                    mc, s_ps = emit_qk_half(pos, 1)
                    emit_exp(pos // NPAIR, mc, s_ps)
                while kproj_done < min(pos // 2 + 3, HW // 512):
                    emit_kproj(kproj_done)
                    kproj_done += 1
                if pos == 13 and qproj_done < 4:
                    emit_qproj(2)
                    emit_qproj(3)
                    qproj_done = 4
                if bg_done < 8 and pos >= 2 * bg_done - 3:
                    eg = bg_front(bg_done)
                    bg_back(bg_done, eg)
                    vt_mms(bg_done)
                    bg_done += 1
                drain_av(pos, 3)
            if DBG:
                nc.sync.dma_start(out=dbg_d["vt"][:], in_=VT[:])
                nc.sync.dma_start(out=dbg_d["lnm"][:], in_=lnm_sb[:])
                nc.gpsimd.dma_start(out=dbg_d["k2"][:], in_=K2[:])
                nc.gpsimd.dma_start(out=dbg_d["q2"][:], in_=Q2[:])

    nc.compile()
    return nc


def prep_inputs(inputs: dict):
    """Host-side packing: returns (in_maps, scalars, shape)."""
    x = np.asarray(inputs["x"], np.float32)        # (B, C, H, W)
    Bsz = x.shape[0]
    Wq = np.asarray(inputs["Wq"], np.float32)
    Wk = np.asarray(inputs["Wk"], np.float32)
    Wv = np.asarray(inputs["Wv"], np.float32)
    We1 = np.asarray(inputs["We1"], np.float32)    # (CH, C, 3, 3)
    be1 = np.asarray(inputs["be1"], np.float32)
    bn_w = np.asarray(inputs["bn_w"], np.float32)
    bn_b = np.asarray(inputs["bn_b"], np.float32)
    bn_mean = np.asarray(inputs["bn_mean"], np.float32)
    bn_var = np.asarray(inputs["bn_var"], np.float32)
    We2 = np.asarray(inputs["We2"], np.float32)    # (1, CH)
    be2 = float(np.asarray(inputs["be2"]).reshape(-1)[0])
    gamma = float(np.asarray(inputs["gamma"]).reshape(-1)[0])
    beta = float(np.asarray(inputs["beta"]).reshape(-1)[0])
    assert abs(beta) < 0.999, "kernel assumes 1 + beta*edge > 0"

    bn_s = bn_w / np.sqrt(bn_var + BN_EPS)
    We1s = We1 * bn_s[:, None, None, None]
    bn_t = (be1 - bn_mean) * bn_s + bn_b

    # A rows 0-63 hold x shifted so col f = x[f-1] (left neighbor, kx=0);
    # A rows 64-127 hold x itself (kx=1); B holds x[f+1] (right, kx=2).
    wcatA = np.zeros((128, 96), np.float32)
    wcatB = np.zeros((64, 96), np.float32)
    for ky in range(3):
        wcatA[0:64, 32 * ky:32 * ky + 32] = We1s[:, :, ky, 0].T
        wcatA[64:128, 32 * ky:32 * ky + 32] = We1s[:, :, ky, 1].T
        wcatB[0:64, 32 * ky:32 * ky + 32] = We1s[:, :, ky, 2].T

    we2_t = np.repeat(We2.reshape(1, CH).T, 2, axis=1)

    shared = {
        "wq_t": np.ascontiguousarray(Wq.T),
        "wk_t": np.ascontiguousarray(Wk.T),
        "wv_bf": np.ascontiguousarray(gamma * Wv.T).astype(ml_dtypes.bfloat16),
        "wcatA": wcatA,
        "wcatB": wcatB,
        "we2_t": np.ascontiguousarray(we2_t),
        "bn_t": bn_t.reshape(CH, 1),
    }
    in_maps = []
    for core in range(N_CORES):
        b, h = core // 2, core % 2
        xb = np.ascontiguousarray(x[b].reshape(C, HW))
        xq = np.ascontiguousarray(xb[:, h * NQ:(h + 1) * NQ])
        # xqT packed: xqt[p, 64k+c] = xq[c, 128k+p]
        xqt = np.ascontiguousarray(
            xq.reshape(C, NCH, 128).transpose(2, 1, 0).reshape(128, NCH * C))
        m = dict(shared)
        m["x"] = xb
        m["x_bf"] = xb.astype(ml_dtypes.bfloat16)
        m["xq"] = xq
        m["xqt"] = xqt
        in_maps.append(m)
    return in_maps, (beta, gamma, be2), (Bsz, x.shape[2], x.shape[3])


_cache = {}


def get_program(scalars):
    if scalars not in _cache:
        _cache[scalars] = build_program(*scalars)
    return _cache[scalars]


def kernel(**inputs) -> np.ndarray:
    in_maps, scalars, (Bsz, H, W) = prep_inputs(inputs)
    nc = get_program(scalars)
    res = run_bass_kernel_spmd(nc, in_maps, list(range(N_CORES)))
    out = np.empty((Bsz, C, H * W), np.float32)
    for core in range(N_CORES):
        b, h = core // 2, core % 2
        r = np.asarray(res.results[core]["out"])  # [128, NCH*64]
        # out[c, 128k+p] = r[p, 64k+c]
        blk = r.reshape(128, NCH, C).transpose(2, 1, 0).reshape(C, NQ)
        out[b][:, h * NQ:(h + 1) * NQ] = blk
    return out.reshape(Bsz, C, H, W)


# revision 15
# speedup vs baseline: 1.1702x; 1.1702x over previous
"""BoundaryAwareSmoothAttention Trainium2 kernel (v2).

Math (per batch b, HW=4096, C=64):
  Q = Wq x, K = Wk x, V = Wv x                   (1x1 convs, biases zero)
  S[n,m]  = q_n . k_m
  edge[m] = sigmoid(We2 . relu(BN(conv3x3(x))) + be2)
  mod[m]  = 1 + beta*edge[m]
  fa[n,m] = exp(S[n,m] - 32 + ln mod[m])         (mod folded into the exp
                                                  bias; softmax Z cancels in
                                                  the L1 renorm)
  out[c,n] = gamma * (sum_m V[c,m] fa[n,m]) / (sum_m fa[n,m]) + x[c,n]

Sharding: 8 cores = 4 batches x 2 query-halves (n in [h*2048, h*2048+2048)).

v2 vs v1: the AV product is computed TRANSPOSED -- for each 128-query chunk
j, P_j[n,c] = sum_m fa[m,n]*VT[m,c] with fa (bf16) as the stationary
operand, so all 128 psum partitions are live (v1's orientation used 66 of
128): AV drops 65536 -> ~34K PE cycles.  VT holds [gamma*V^T | 1 | 0] in
bf16 and the ones column accumulates the L1 denominator for free.  V^T is
projected in bf16 (1 cy/row at free-dim 64 where f32r pays 4).  The exp
stream is split column-wise across ACT (native exp, psum->bf16) and DVE
(Schraudolph: bf16 bits = rint(C1*S + PB[m]) written as int16), halving the
66us single-engine exp stream and letting the 2-buffer S psum drain fast.
The epilogue is one DVE scalar_tensor_tensor per chunk in the transposed
layout (the denominator is per-partition there); the host untransposes.
A 1x1 warmup matmul at t~0 starts the PE p-state ramp before DMAs land.
"""

import numpy as np
import ml_dtypes

import concourse.bass as bass
import concourse.tile as tile
from concourse import bacc, mybir
from concourse.bass_utils import run_bass_kernel_spmd

F32 = mybir.dt.float32
F32R = mybir.dt.float32r
BF16 = mybir.dt.bfloat16
I16 = mybir.dt.int16
AF = mybir.ActivationFunctionType
ALU = mybir.AluOpType

C = 64
CH = 32
HW = 4096
NQ = 2048   # queries per core
QB = 1024   # queries per position-block
NPAIR = 16  # positions per qb; position p covers m-chunks 2p, 2p+1
NPOS = 32
NCH = 16    # 128-query output chunks per core
N_CORES = 8
SHIFT = 16.0
BN_EPS = 1e-5

# Schraudolph bf16 exp: bits_i16 = rint(SC1*x + SC2), calibrated for
# round-to-nearest f32->i16 conversion (max rel err 3.3%).  The softmax
# shift is 16 (not the row max): it cancels in the L1 renorm, and S-16 in
# [-86, +54] keeps the int16 bits in [0, 32767] and e^(S-16) finite in
# bf16 for any |S| <= ~70.
LOG2E = 1.4426950408889634
SC1 = 128.0 * LOG2E
SC2 = 128.0 * (127.0 - 0.044)

CFG = {
    "lag": 3,        # AV trails QK by this many positions
    "dve_tiles": 30,  # of the 64 exp tiles, how many go to DVE (Schraudolph)
    "fa_bufs": 12,
}


def build_program(beta: float, gamma: float, be2: float):
    nc = bacc.Bacc("TRN2", target_bir_lowering=False, debug=False,
                   num_devices=N_CORES)

    def din(name, shape, dt=F32):
        return nc.dram_tensor(name, shape, dt, kind="ExternalInput").ap()

    x_d = din("x", [C, HW])
    xbf_d = din("x_bf", [C, HW], BF16)
    xq_d = din("xq", [C, NQ])
    xqt_d = din("xqt", [128, NCH * 64])
    wq_d = din("wq_t", [C, C])
    wk_d = din("wk_t", [C, C])
    wv_d = din("wv_bf", [C, C], BF16)     # gamma * Wv^T, bf16
    wcatA_d = din("wcatA", [128, 96])
    wcatB_d = din("wcatB", [64, 96])
    we2_d = din("we2_t", [CH, 2])
    bnt_d = din("bn_t", [CH, 1])
    out_d = nc.dram_tensor("out", [128, NCH * 64], F32,
                           kind="ExternalOutput").ap()
    import os
    DBG = os.environ.get("KDBG", "")
    dbg_d = {}
    if DBG:
        dbg_d["vt"] = nc.dram_tensor("dbg_vt", [128, 66 * 32], BF16,
                                     kind="ExternalOutput").ap()
        dbg_d["lnm"] = nc.dram_tensor("dbg_lnm", [128, 32], F32,
                                      kind="ExternalOutput").ap()
        dbg_d["fa0"] = nc.dram_tensor("dbg_fa0", [128, QB], BF16,
                                      kind="ExternalOutput").ap()
        dbg_d["fa1"] = nc.dram_tensor("dbg_fa1", [128, QB], BF16,
                                      kind="ExternalOutput").ap()
        dbg_d["av0"] = nc.dram_tensor("dbg_av0", [128, 264], F32,
                                      kind="ExternalOutput").ap()
        dbg_d["av3"] = nc.dram_tensor("dbg_av3", [128, 264], F32,
                                      kind="ExternalOutput").ap()
        dbg_d["fa14"] = nc.dram_tensor("dbg_fa14", [128, QB], BF16,
                                       kind="ExternalOutput").ap()
        dbg_d["k2"] = nc.dram_tensor("dbg_k2", [128, HW], F32,
                                     kind="ExternalOutput").ap()
        dbg_d["q2"] = nc.dram_tensor("dbg_q2", [128, NQ], F32,
                                     kind="ExternalOutput").ap()

    LAG = CFG["lag"]

    with tile.TileContext(nc) as tc:
        with (
            tc.tile_pool(name="consts", bufs=1) as consts,
            tc.tile_pool(name="bigs", bufs=1) as bigs,
            tc.tile_pool(name="fa_p", bufs=CFG["fa_bufs"]) as fa_p,
            tc.tile_pool(name="ep", bufs=4) as ep,
            tc.tile_pool(name="ps_s", bufs=2, space="PSUM") as ps_s,
            tc.tile_pool(name="ps_av", bufs=2, space="PSUM") as ps_av,
            tc.tile_pool(name="ps_bg", bufs=2, space="PSUM") as ps_bg,
        ):
            # ---- big SBUF tensors -----------------------------------------
            A = bigs.tile([128, HW], F32R)       # [x_m1 ; x]
            B = bigs.tile([64, HW], F32R)        # x_p1
            x_r = bigs.tile([C, HW], F32R)       # K proj rhs
            x_bf = bigs.tile([C, HW], BF16)      # V^T proj lhsT
            xq_r = bigs.tile([C, NQ], F32R)      # Q proj rhs
            xqT = bigs.tile([128, NCH * 64], F32)
            K2 = bigs.tile([128, HW], F32R)      # K duplicated on both halves
            Q2 = bigs.tile([128, NQ], F32R)      # Q duplicated on both halves
            VT = bigs.tile([128, 66 * 32], BF16)  # chunks [gamma*V^T | 1 | 0]
            relu_sb = bigs.tile([CH, HW], F32R)
            out_sb = bigs.tile([128, NCH * 64], F32)
            VT_v = VT[:].rearrange("p (j w) -> p j w", w=66)

            # ---- constant/small tiles -------------------------------------
            wq_r = consts.tile([C, C], F32R)
            wk_r = consts.tile([C, C], F32R)
            wv_b = consts.tile([C, C], BF16)
            wcatA_r = consts.tile([128, 96], F32R)
            wcatB_r = consts.tile([64, 96], F32R)
            we2_r = consts.tile([CH, 2], F32R)
            bnt_sb = consts.tile([CH, 1], F32)
            ones_b = consts.tile([128, 1], F32)
            opb_b = consts.tile([128, 1], F32)
            ones32 = consts.tile([128, 32], F32)
            z32 = consts.tile([128, 32], F32)
            z64 = consts.tile([64, 64], F32)
            warm = consts.tile([1, 1], BF16)
            lnm_sb = consts.tile([128, 32], F32)   # ln(mod) per m-chunk col
            bia_sb = consts.tile([128, 32], F32)   # lnm - 32 (ACT exp bias)
            pb_sb = consts.tile([128, 32], F32)    # SC1*(lnm-32)+SC2 (DVE)

            # warmup matmul to start the PE p-state ramp at t~0
            nc.vector.memset(warm[:], 0.0)
            wps = ps_bg.tile([1, 1], F32, tag="bg")
            nc.tensor.matmul(wps[:], warm[:], warm[:], start=True, stop=True)

            # input DMAs, QK/conv path first
            wk_f = consts.tile([C, C], F32)
            wq_f = consts.tile([C, C], F32)
            nc.sync.dma_start(out=wk_f[:], in_=wk_d[:])
            nc.sync.dma_start(out=wq_f[:], in_=wq_d[:])
            nc.vector.tensor_copy(wk_r[:], wk_f[:])
            nc.vector.tensor_copy(wq_r[:], wq_f[:])
            nc.gpsimd.dma_start(out=x_r[:, 0:1152], in_=x_d[:, 0:1152])
            nc.gpsimd.dma_start(out=xq_r[:, 0:1024], in_=xq_d[:, 0:1024])
            nc.gpsimd.dma_start(out=A[64:128, 0:1152], in_=x_d[:, 0:1152])
            nc.gpsimd.dma_start(out=wcatA_r[:], in_=wcatA_d[:])
            nc.gpsimd.dma_start(out=wcatB_r[:], in_=wcatB_d[:])
            nc.gpsimd.dma_start(out=we2_r[:], in_=we2_d[:])
            nc.sync.dma_start(out=bnt_sb[:], in_=bnt_d[:])
            nc.gpsimd.dma_start(out=xq_r[:, 1024:NQ], in_=xq_d[:, 1024:NQ])
            nc.gpsimd.dma_start(out=x_r[:, 1152:HW], in_=x_d[:, 1152:HW])
            nc.gpsimd.dma_start(out=A[64:128, 1152:HW], in_=x_d[:, 1152:HW])
            nc.scalar.dma_start(out=wv_b[:], in_=wv_d[:])
            nc.scalar.dma_start(out=x_bf[:, 0:1024], in_=xbf_d[:, 0:1024])
            nc.scalar.dma_start(out=x_bf[:, 1024:HW], in_=xbf_d[:, 1024:HW])
            nc.scalar.dma_start(out=xqT[:], in_=xqt_d[:])

            nc.vector.memset(ones_b[:], 1.0)
            nc.vector.memset(opb_b[:], 1.0 + beta)
            nc.vector.memset(ones32[:], 1.0)
            nc.vector.memset(z32[:], 0.0)
            nc.vector.memset(z64[:], 0.0)
            # VT col 64 = 1 (denominator), col 65 = 0 (pad)
            nc.vector.tensor_copy(VT_v[:, :, 64], ones32[:])
            nc.vector.tensor_copy(VT_v[:, :, 65], z32[:])

            # ---- Q/K projections ------------------------------------------
            def emit_kproj(t):
                kp = ps_bg.tile([64, 512], F32, tag="bg")
                nc.tensor.matmul(kp[:], wk_r[:], x_r[:, 512 * t:512 * t + 512],
                                 start=True, stop=True)
                nc.vector.tensor_copy(K2[0:64, 512 * t:512 * t + 512], kp[:])
                nc.sync.dma_start(out=K2[64:128, 512 * t:512 * t + 512],
                                  in_=K2[0:64, 512 * t:512 * t + 512])

            def emit_qproj(t):
                qp = ps_bg.tile([64, 512], F32, tag="bg")
                nc.tensor.matmul(qp[:], wq_r[:], xq_r[:, 512 * t:512 * t + 512],
                                 start=True, stop=True)
                nc.vector.tensor_copy(Q2[0:64, 512 * t:512 * t + 512], qp[:])
                nc.sync.dma_start(out=Q2[64:128, 512 * t:512 * t + 512],
                                  in_=Q2[0:64, 512 * t:512 * t + 512])

            # ---- edge/background pipeline ---------------------------------
            A_vw = A[0:64, :].rearrange("p (y x) -> p y x", x=64)
            B_vw = B[0:64, :].rearrange("p (y x) -> p y x", x=64)

            def bg_prep(t):
                # x_m1 (A rows 0-63) / x_p1 (B) slices this conv tile reads,
                # plus their SAME-pad zeros; on Pool to keep DVE free
                r0 = 0 if t == 0 else 512 * t + 576
                r1 = min(512 * t + 1088, HW)
                if r1 <= r0:
                    return
                a0 = max(r0, 1)
                nc.gpsimd.tensor_copy(A[0:64, a0:r1], x_r[:, a0 - 1:r1 - 1])
                nc.gpsimd.tensor_copy(B[0:64, r0:r1 - 1], x_r[:, r0 + 1:r1])
                y0, y1 = r0 // 64, r1 // 64
                nc.gpsimd.tensor_copy(A_vw[:, y0:y1, 0], z64[:, 0:y1 - y0])
                nc.gpsimd.tensor_copy(B_vw[:, y0:y1, 63], z64[:, 0:y1 - y0])

            def bg_front(t):
                bg_prep(t)
                t0 = 512 * t
                ep_ps = ps_bg.tile([CH, 512], F32, tag="bg")
                mms = []
                for dy in (0, -1, 1):
                    lo = max(t0, -64 * dy)
                    hi = min(t0 + 512, HW - max(0, 64 * dy))
                    if hi <= lo:
                        continue
                    sl_out = ep_ps[:, lo - t0:hi - t0]
                    ky = dy + 1
                    mms.append((sl_out, wcatA_r[:, 32 * ky:32 * ky + 32],
                                A[:, lo + 64 * dy:hi + 64 * dy]))
                    mms.append((sl_out, wcatB_r[:, 32 * ky:32 * ky + 32],
                                B[0:64, lo + 64 * dy:hi + 64 * dy]))
                for i, (o, l, r) in enumerate(mms):
                    nc.tensor.matmul(o, l, r, start=(i == 0),
                                     stop=(i == len(mms) - 1),
                                     skip_group_check=True)
                nc.scalar.activation(relu_sb[:, t0:t0 + 512], ep_ps[:],
                                     AF.Relu, bias=bnt_sb[:, 0:1], scale=1.0)
                eg_t = ps_bg.tile([128, 8], F32, tag="bg")
                for jj in range(4):
                    j = 4 * t + jj
                    nc.tensor.matmul(eg_t[:, 2 * jj:2 * jj + 2],
                                     relu_sb[:, 128 * j:128 * j + 128],
                                     we2_r[:], start=True, stop=True,
                                     skip_group_check=True)
                return eg_t

            def bg_back(t, eg_t):
                # ln(mod) = ln(1+beta*sigmoid(z+be2)) = Ln(u+1+beta) - Ln(u+1)
                # with u = Exp(-(z+be2)): keeps every ACT function in the
                # natural_log_exp_and_others table set (no table reloads)
                g = slice(4 * t, 4 * t + 4)
                u = ep.tile([128, 4], F32, tag="sg")
                la = ep.tile([128, 4], F32, tag="sg")
                lb = ep.tile([128, 4], F32, tag="sg")
                nc.scalar.activation(u[:], eg_t[:, 0:8:2], AF.Exp,
                                     bias=-be2, scale=-1.0)
                nc.scalar.activation(la[:], u[:], AF.Ln,
                                     bias=opb_b[:, 0:1], scale=1.0)
                nc.scalar.activation(lb[:], u[:], AF.Ln,
                                     bias=ones_b[:, 0:1], scale=1.0)
                nc.gpsimd.tensor_sub(lnm_sb[:, g], la[:], lb[:])
                nc.gpsimd.tensor_scalar_add(bia_sb[:, g], lnm_sb[:, g], -SHIFT)
                nc.gpsimd.tensor_scalar(out=pb_sb[:, g], in0=lnm_sb[:, g],
                                        scalar1=SC1,
                                        scalar2=SC2 - SHIFT * SC1,
                                        op0=ALU.mult, op1=ALU.add)

            def vt_mms(t):
                # gamma*V^T for m-chunks 4t..4t+3, bf16 (full rate at free=64)
                j0 = 4 * t
                vp = ps_bg.tile([128, 256], F32, tag="bg")
                for jj in range(4):
                    nc.tensor.matmul(vp[:, 64 * jj:64 * jj + 64],
                                     x_bf[:, 128 * (j0 + jj):128 * (j0 + jj) + 128],
                                     wv_b[:], start=True, stop=True,
                                     skip_group_check=True)
                vp_v = vp[:].rearrange("p (j w) -> p j w", w=64)
                nc.scalar.activation(VT_v[:, j0:j0 + 4, 0:64], vp_v[:],
                                     AF.Copy)

            # ---- QK + split exp -------------------------------------------
            fa_store = {}

            def emit_qk_half(pos, k):
                qb, pair = pos // NPAIR, pos % NPAIR
                q0 = QB * qb
                mc = 2 * pair + k
                r0, r1 = (0, 64) if k == 0 else (64, 128)
                s_ps = ps_s.tile([128, QB], F32, tag="s")
                for h in range(QB // 512):
                    nc.tensor.matmul(
                        s_ps[:, 512 * h:512 * h + 512],
                        K2[r0:r1, 128 * mc:128 * mc + 128],
                        Q2[r0:r1, q0 + 512 * h:q0 + 512 * h + 512],
                        start=True, stop=True)
                return mc, s_ps

            # whole-tile engine assignment: DVE takes Schraudolph tiles at a
            # steady cadence, ACT the rest (native exp); both use shift 16
            # so a query's weights stay mutually consistent.
            ND = CFG["dve_tiles"]

            def exp_on_dve(qb, pair, k):
                i = 32 * qb + 2 * pair + k
                return (i * ND) // 64 != ((i + 1) * ND) // 64

            def emit_exp(qb, mc, s_ps):
                fa = fa_p.tile([128, QB], BF16, tag="fa")
                if exp_on_dve(qb, mc // 2, mc % 2):
                    nc.vector.tensor_scalar(
                        out=fa[:].bitcast(I16), in0=s_ps[:],
                        scalar1=SC1, scalar2=pb_sb[:, mc:mc + 1],
                        op0=ALU.mult, op1=ALU.add)
                else:
                    nc.scalar.activation(fa[:], s_ps[:], AF.Exp,
                                         bias=bia_sb[:, mc:mc + 1], scale=1.0)
                if DBG and qb == 0 and mc in (0, 1):
                    nc.sync.dma_start(out=dbg_d["fa" + str(mc)][:], in_=fa[:])
                fa_store[(qb, mc)] = fa

            # ---- AV (transposed) + epilogue -------------------------------
            av_tiles = {}

            def emit_av_half(pos_done, half):
                # apply fa pair (pos_done) to 4 of the 8 query-chunk
                # accumulators of its qb
                qb, pair = pos_done // NPAIR, pos_done % NPAIR
                if pair == 0 and half == 0:
                    tA = ps_av.tile([128, 264], F32, tag="av", name=f"avA{qb}")
                    tB = ps_av.tile([128, 264], F32, tag="av", name=f"avB{qb}")
                    # a start=True matmul would wipe the other in-flight
                    # accumulators sharing the bank, so zero via ACT and
                    # accumulate with start=False throughout
                    nc.scalar.memzero(tA[:])
                    nc.scalar.memzero(tB[:])
                    av_tiles[qb] = (tA, tB)
                t = av_tiles[qb][half]
                for jj in range(4):
                    j = 4 * half + jj
                    for mc in (2 * pair, 2 * pair + 1):
                        fa = fa_store[(qb, mc)]
                        nc.tensor.matmul(
                            t[:, 66 * jj:66 * jj + 66],
                            fa[:, 128 * j:128 * j + 128],
                            VT[:, 66 * mc:66 * mc + 66],
                            start=False, stop=(mc == 31),
                            skip_group_check=True)
                if half == 1:
                    del fa_store[(qb, 2 * pair)]
                    del fa_store[(qb, 2 * pair + 1)]

            def epilogue_half(qb, half):
                t = av_tiles[qb][half]
                if DBG and qb == 0 and half == 0:
                    avs = ep.tile([128, 264], F32, tag="dbgav")
                    nc.vector.tensor_copy(avs[:], t[:])
                    nc.sync.dma_start(out=dbg_d["av0"][:], in_=avs[:])
                if DBG and qb == 1 and half == 1:
                    avs = ep.tile([128, 264], F32, tag="dbgav")
                    nc.vector.tensor_copy(avs[:], t[:])
                    nc.sync.dma_start(out=dbg_d["av3"][:], in_=avs[:])
                tv = t[:].rearrange("p (j w) -> p j w", w=66)
                rc = ep.tile([128, 4], F32, tag="rc")
                nc.vector.reciprocal(rc[:], tv[:, :, 64])
                for jj in range(4):
                    k = 8 * qb + 4 * half + jj
                    nc.vector.scalar_tensor_tensor(
                        out=out_sb[:, 64 * k:64 * k + 64],
                        in0=tv[:, jj, 0:64], scalar=rc[:, jj:jj + 1],
                        in1=xqT[:, 64 * k:64 * k + 64],
                        op0=ALU.mult, op1=ALU.add)
                k0 = 8 * qb + 4 * half
                nc.sync.dma_start(out=out_d[:, 64 * k0:64 * k0 + 256],
                                  in_=out_sb[:, 64 * k0:64 * k0 + 256])

            # ---- main loop ------------------------------------------------
            # bg tile t must be ready before pos 2t (bias cols 4t..4t+3).
            eg0 = bg_front(0)
            bg_back(0, eg0)
            vt_mms(0)
            emit_kproj(0)
            emit_qproj(0)
            emit_qproj(1)
            eg1 = bg_front(1)
            bg_back(1, eg1)
            vt_mms(1)
            kproj_done, qproj_done, bg_done = 1, 2, 2
            av_next = 0

            def drain_av(pos, budget):
                nonlocal av_next
                while budget > 0 and av_next // 2 <= pos - LAG \
                        and av_next < 2 * NPOS:
                    emit_av_half(av_next // 2, av_next % 2)
                    if av_next % (2 * NPAIR) == 2 * NPAIR - 1:
                        qb = av_next // (2 * NPAIR)
                        epilogue_half(qb, 0)
                        epilogue_half(qb, 1)
                    av_next += 1
                    budget -= 1

            for pos in range(NPOS + LAG + 1):
                if pos < NPOS:
                    mc, s_ps = emit_qk_half(pos, 0)
                    emit_exp(pos // NPAIR, mc, s_ps)
                drain_av(pos, 1)
                if pos < NPOS:
                    mc, s_ps = emit_qk_half(pos, 1)
                    emit_exp(pos // NPAIR, mc, s_ps)
                while kproj_done < min(pos // 2 + 3, HW // 512):
                    emit_kproj(kproj_done)
                    kproj_done += 1
                if pos == 13 and qproj_done < 4:
                    emit_qproj(2)
                    emit_qproj(3)
                    qproj_done = 4
                if bg_done < 8 and pos >= 2 * bg_done - 3:
                    eg = bg_front(bg_done)
                    bg_back(bg_done, eg)
                    vt_mms(bg_done)
                    bg_done += 1
                drain_av(pos, 3)
            if DBG:
                nc.sync.dma_start(out=dbg_d["vt"][:], in_=VT[:])
                nc.sync.dma_start(out=dbg_d["lnm"][:], in_=lnm_sb[:])
                nc.gpsimd.dma_start(out=dbg_d["k2"][:], in_=K2[:])
                nc.gpsimd.dma_start(out=dbg_d["q2"][:], in_=Q2[:])

    nc.compile()
    return nc


def prep_inputs(inputs: dict):
    """Host-side packing: returns (in_maps, scalars, shape)."""
    x = np.asarray(inputs["x"], np.float32)        # (B, C, H, W)
    Bsz = x.shape[0]
    Wq = np.asarray(inputs["Wq"], np.float32)
    Wk = np.asarray(inputs["Wk"], np.float32)
    Wv = np.asarray(inputs["Wv"], np.float32)
    We1 = np.asarray(inputs["We1"], np.float32)    # (CH, C, 3, 3)
    be1 = np.asarray(inputs["be1"], np.float32)
    bn_w = np.asarray(inputs["bn_w"], np.float32)
    bn_b = np.asarray(inputs["bn_b"], np.float32)
    bn_mean = np.asarray(inputs["bn_mean"], np.float32)
    bn_var = np.asarray(inputs["bn_var"], np.float32)
    We2 = np.asarray(inputs["We2"], np.float32)    # (1, CH)
    be2 = float(np.asarray(inputs["be2"]).reshape(-1)[0])
    gamma = float(np.asarray(inputs["gamma"]).reshape(-1)[0])
    beta = float(np.asarray(inputs["beta"]).reshape(-1)[0])
    assert abs(beta) < 0.999, "kernel assumes 1 + beta*edge > 0"

    bn_s = bn_w / np.sqrt(bn_var + BN_EPS)
    We1s = We1 * bn_s[:, None, None, None]
    bn_t = (be1 - bn_mean) * bn_s + bn_b

    # A rows 0-63 hold x shifted so col f = x[f-1] (left neighbor, kx=0);
    # A rows 64-127 hold x itself (kx=1); B holds x[f+1] (right, kx=2).
    wcatA = np.zeros((128, 96), np.float32)
    wcatB = np.zeros((64, 96), np.float32)
    for ky in range(3):
        wcatA[0:64, 32 * ky:32 * ky + 32] = We1s[:, :, ky, 0].T
        wcatA[64:128, 32 * ky:32 * ky + 32] = We1s[:, :, ky, 1].T
        wcatB[0:64, 32 * ky:32 * ky + 32] = We1s[:, :, ky, 2].T

    we2_t = np.repeat(We2.reshape(1, CH).T, 2, axis=1)

    shared = {
        "wq_t": np.ascontiguousarray(Wq.T),
        "wk_t": np.ascontiguousarray(Wk.T),
        "wv_bf": np.ascontiguousarray(gamma * Wv.T).astype(ml_dtypes.bfloat16),
        "wcatA": wcatA,
        "wcatB": wcatB,
        "we2_t": np.ascontiguousarray(we2_t),
        "bn_t": bn_t.reshape(CH, 1),
    }
    in_maps = []
    for core in range(N_CORES):
        b, h = core // 2, core % 2
        xb = np.ascontiguousarray(x[b].reshape(C, HW))
        xq = np.ascontiguousarray(xb[:, h * NQ:(h + 1) * NQ])
        # xqT packed: xqt[p, 64k+c] = xq[c, 128k+p]
        xqt = np.ascontiguousarray(
            xq.reshape(C, NCH, 128).transpose(2, 1, 0).reshape(128, NCH * C))
        m = dict(shared)
        m["x"] = xb
        m["x_bf"] = xb.astype(ml_dtypes.bfloat16)
        m["xq"] = xq
        m["xqt"] = xqt
        in_maps.append(m)
    return in_maps, (beta, gamma, be2), (Bsz, x.shape[2], x.shape[3])


_cache = {}


def get_program(scalars):
    if scalars not in _cache:
        _cache[scalars] = build_program(*scalars)
    return _cache[scalars]


def kernel(**inputs) -> np.ndarray:
    in_maps, scalars, (Bsz, H, W) = prep_inputs(inputs)
    nc = get_program(scalars)
    res = run_bass_kernel_spmd(nc, in_maps, list(range(N_CORES)))
    out = np.empty((Bsz, C, H * W), np.float32)
    for core in range(N_CORES):
        b, h = core // 2, core % 2
        r = np.asarray(res.results[core]["out"])  # [128, NCH*64]
        # out[c, 128k+p] = r[p, 64k+c]
        blk = r.reshape(128, NCH, C).transpose(2, 1, 0).reshape(C, NQ)
        out[b][:, h * NQ:(h + 1) * NQ] = blk
    return out.reshape(Bsz, C, H, W)
